# revision 1
# baseline (speedup 1.0000x reference)
"""Trainium2 Bass kernel for CustomPatchEmbedding (ragged patch gather + two projections).

Strategy (data-parallel over batch, 8 cores x 4 images):
  - Images are repacked on host into a sliding 16-row-block channel-last
    layout: blk[y, x, dy, c] = img[c, y+dy, x] (16x redundant, bf16). A fine
    16x16 patch is then ONE contiguous 768-element run (block y); a coarse
    64x64 patch is FOUR contiguous 3072-element runs. The HW indirect DMA
    consumes exactly one offset per destination partition, so each gather
    instruction moves 128 patch-runs; the whole gather is 12 instructions
    (the naive per-row gather needs 576, at ~1us of GpSimd SWDGE time each).
  - Weights are cast to bf16 and column-permuted on host to match the
    (block, dx, dy, c) feature order; PSUM accumulates fp32.
  - TensorE transposes 128-feature chunks to [feature, patch] (4 chunks per
    PSUM tile, one DVE copy per tile), then accumulates bf16 matmuls into
    PSUM [patch, 256] fp32. Compute is emitted software-pipelined (the next
    set's transposes before this set's matmuls) and coarse/fine sets are
    interleaved to match gather arrival order, so the PE never waits on DVE.
  - Fine weights are resident in SBUF; coarse weights stream as 12 large
    [128, 2048] DMAs.

kernel(**inputs) takes the FULL unsharded inputs and returns (32, 288, 256) f32.
"""
import sys
import numpy as np

sys.path.insert(0, "/opt/trn_rl_repo")

import ml_dtypes
import concourse.bass as bass
import concourse.bacc as bacc
import concourse.mybir as mybir
import concourse.tile as tile
from concourse.bass_utils import run_bass_kernel_spmd
from contextlib import ExitStack

# Problem constants (hardcoded per spec).
B, C, H, W = 32, 3, 512, 512
FP, CP = 16, 64
NF, NCO = 256, 32
D = 256
NCORES = 8
IPC = B // NCORES              # images per core
KF = C * FP * FP               # 768  fine features
KC = C * CP * CP               # 12288 coarse features
P = 128
GF = IPC * 2                   # fine groups of 128 patches per core
NKF = KF // P                  # 6 fine k-chunks
NKC = KC // P                  # 96 coarse k-chunks

R = 16                         # rows packed per block in the sliding layout
RUN_F = FP * R * C             # 768  elements per fine gather run (whole patch)
RUN_C = CP * R * C             # 3072 elements per coarse gather run
NBF = FP // R                  # 1 run per fine patch
NBC = CP // R                  # 4 runs per coarse patch
BLK_STRIDE = W * R * C         # 24576 elements per block row
IMG8 = H * BLK_STRIDE          # padded per-image element count
NFLAT8 = IPC * IMG8

FDT = mybir.dt.float32
BDT = mybir.dt.bfloat16
IDT = mybir.dt.int32
BF16 = ml_dtypes.bfloat16


def _emit(nc, tc, t):
    """Emit the per-core Tile program. `t` maps tensor name -> dram handle."""
    with ExitStack() as ctx:
        const = ctx.enter_context(tc.tile_pool(name="const", bufs=1))
        gf_pool = ctx.enter_context(tc.tile_pool(name="gf", bufs=GF))
        gc_pool = ctx.enter_context(tc.tile_pool(name="gc", bufs=2))
        wc_pool = ctx.enter_context(tc.tile_pool(name="wc", bufs=4))
        lt_pool = ctx.enter_context(tc.tile_pool(name="lt", bufs=4))
        ob_pool = ctx.enter_context(tc.tile_pool(name="ob", bufs=3))
        ps_tp = ctx.enter_context(tc.tile_pool(name="ps_tp", bufs=3, space="PSUM"))
        ps_f = ctx.enter_context(tc.tile_pool(name="ps_f", bufs=2, space="PSUM"))
        ps_c = ctx.enter_context(tc.tile_pool(name="ps_c", bufs=1, space="PSUM"))

        # --- offsets first so gathers can start immediately ---
        fidx = const.tile([P, GF * NBF], IDT)
        nc.sync.dma_start(fidx[:], t["fidx"][:])
        cidx = const.tile([P, NBC], IDT)
        nc.sync.dma_start(cidx[:], t["cidx"][:])
        identity = const.tile([P, P], BDT)
        nc.sync.dma_start(identity[:], t["ident"][:])
        bias_f = const.tile([P, D], FDT)
        nc.sync.dma_start(bias_f[:], t["bias_f"][:])
        bias_c = const.tile([P, D], FDT)
        nc.sync.dma_start(bias_c[:], t["bias_c"][:])
        wf = const.tile([P, NKF * D], BDT)
        nc.sync.dma_start(wf[:], t["wf2"][:])

        imgs8 = t["imgs8"]
        out = t["out"]

        # --- gathers: coarse (2 half-tiles x 2 runs), then fine (8 groups) ---
        gcs = []
        for half in range(2):
            gt = gc_pool.tile([P, 2 * RUN_C], BDT, tag="gc")
            for kbl in range(2):
                kb = half * 2 + kbl
                nc.gpsimd.indirect_dma_start(
                    out=gt[:, kbl * RUN_C:(kbl + 1) * RUN_C], out_offset=None,
                    in_=imgs8[:],
                    in_offset=bass.IndirectOffsetOnAxis(ap=cidx[:, kb:kb + 1], axis=0),
                )
            gcs.append(gt)
        gfs = []
        for g in range(GF):
            gt = gf_pool.tile([P, RUN_F], BDT, tag="gf")
            nc.gpsimd.indirect_dma_start(
                out=gt[:], out_offset=None, in_=imgs8[:],
                in_offset=bass.IndirectOffsetOnAxis(ap=fidx[:, g:g + 1], axis=0),
            )
            gfs.append(gt)

        # --- build the interleaved, software-pipelined compute set list ---
        # Each set: (src_tile, col0, cnt, psum, wsrc, wcol0, start?, stop?, post)
        psum_c = ps_c.tile([P, D], FDT)
        fine_psum = {}
        sets = []

        def coarse_sets(half):
            for sc in range(6):
                s = half * 6 + sc
                wc = wc_pool.tile([P, 8 * D], BDT, tag="wc")
                nc.sync.dma_start(wc[:], t["wc2"][:, s * 8 * D:(s + 1) * 8 * D])
                for st in range(2):
                    k0 = s * 8 + st * 4
                    post = _coarse_post if k0 + 4 == NKC else None
                    sets.append((gcs[half], (sc * 8 + st * 4) % 48 * P, 4,
                                 psum_c, wc, st * 4 * D, k0 == 0, k0 + 4 == NKC, post))

        def _coarse_post():
            oc = ob_pool.tile([P, D], FDT, tag="ob")
            nc.vector.tensor_tensor(
                out=oc[:], in0=psum_c[:], in1=bias_c[:], op=mybir.AluOpType.add
            )
            for b in range(IPC):
                nc.scalar.dma_start(
                    out[b * (NF + NCO) + NF:b * (NF + NCO) + NF + NCO, :],
                    oc[b * NCO:(b + 1) * NCO, :],
                )

        def fine_sets(g):
            b, hh = divmod(g, 2)
            psum = ps_f.tile([P, D], FDT)
            fine_psum[g] = psum

            def post():
                ob = ob_pool.tile([P, D], FDT, tag="ob")
                nc.vector.tensor_tensor(
                    out=ob[:], in0=psum[:], in1=bias_f[:], op=mybir.AluOpType.add
                )
                row0 = b * (NF + NCO) + hh * P
                nc.scalar.dma_start(out[row0:row0 + P, :], ob[:])

            sets.append((gfs[g], 0, 4, psum, wf, 0, True, False, None))
            sets.append((gfs[g], 4 * P, 2, psum, wf, 4 * D, False, True, post))

        coarse_sets(0)
        for g in range(2):
            fine_sets(g)
        coarse_sets(1)
        for g in range(2, GF):
            fine_sets(g)

        # --- emit with 1-set software pipelining: T(s+1) before M(s) ---
        def emit_T(s):
            src, col0, cnt, _, _, _, _, _, _ = s
            tp = ps_tp.tile([P, 512], BDT, tag="tp")
            for i in range(cnt):
                nc.tensor.transpose(
                    out=tp[:, i * P:(i + 1) * P],
                    in_=src[:, col0 + i * P:col0 + (i + 1) * P],
                    identity=identity[:],
                )
            lt = lt_pool.tile([P, 512], BDT, tag="lt")
            nc.vector.tensor_copy(lt[:, 0:cnt * P], tp[:, 0:cnt * P])
            return lt

        def emit_M(s, lt):
            _, _, cnt, psum, wsrc, wcol0, k_start, k_stop, post = s
            for i in range(cnt):
                nc.tensor.matmul(
                    out=psum[:], lhsT=lt[:, i * P:(i + 1) * P],
                    rhs=wsrc[:, wcol0 + i * D:wcol0 + (i + 1) * D],
                    start=(k_start and i == 0), stop=(k_stop and i == cnt - 1),
                )
            if post is not None:
                post()

        prev = None
        for s in sets:
            lt = emit_T(s)
            if prev is not None:
                emit_M(*prev)
            prev = (s, lt)
        emit_M(*prev)


def build(reps: int = 1):
    nc = bacc.Bacc("TRN2", target_bir_lowering=False, debug=False)
    t = {
        "imgs8": nc.dram_tensor("imgs8", [NFLAT8, 1], BDT, kind="ExternalInput"),
        "wf2": nc.dram_tensor("wf2", [P, NKF * D], BDT, kind="ExternalInput"),
        "wc2": nc.dram_tensor("wc2", [P, NKC * D], BDT, kind="ExternalInput"),
        "bias_f": nc.dram_tensor("bias_f", [P, D], FDT, kind="ExternalInput"),
        "bias_c": nc.dram_tensor("bias_c", [P, D], FDT, kind="ExternalInput"),
        "ident": nc.dram_tensor("ident", [P, P], BDT, kind="ExternalInput"),
        "fidx": nc.dram_tensor("fidx", [P, GF * NBF], IDT, kind="ExternalInput"),
        "cidx": nc.dram_tensor("cidx", [P, NBC], IDT, kind="ExternalInput"),
        "out": nc.dram_tensor("out", [IPC * (NF + NCO), D], FDT, kind="ExternalOutput"),
    }
    with tile.TileContext(nc) as tc:
        for _ in range(reps):
            _emit(nc, tc, t)
    nc.compile()
    return nc


def repack_images(images):
    """[B, C, H, W] f32 -> sliding R-row-block channel-last bf16.

    blk[b, y, x, dy, c] = images[b, c, y+dy, x]; y padded to H blocks.
    """
    cl = np.ascontiguousarray(images.transpose(0, 2, 3, 1)).astype(BF16)  # [B, y, x, c]
    sw = np.lib.stride_tricks.sliding_window_view(cl, R, axis=1)  # [B, H-R+1, x, c, dy]
    sw = sw.transpose(0, 1, 2, 4, 3)                              # [B, blk, x, dy, c]
    blob = np.zeros((images.shape[0], H, W, R, C), dtype=BF16)
    blob[:, :H - R + 1] = sw
    return blob


def host_indices(fine_xy, coarse_xy):
    """Element offsets into the per-core imgs8 blob (one per gather run)."""
    kb_f = np.arange(NBF) * R
    base_f = (fine_xy[:, :, 1][..., None] + kb_f) * BLK_STRIDE \
        + fine_xy[:, :, 0][..., None] * (R * C) \
        + (np.arange(IPC) * IMG8)[:, None, None]                  # [IPC, NF, NBF]
    fidx = base_f.reshape(GF, P, NBF).transpose(1, 0, 2).reshape(P, GF * NBF)

    kb_c = np.arange(NBC) * R
    cidx = (coarse_xy[:, :, 1][..., None] + kb_c) * BLK_STRIDE \
        + coarse_xy[:, :, 0][..., None] * (R * C) \
        + (np.arange(IPC) * IMG8)[:, None, None]                  # [IPC, NCO, NBC]
    cidx = cidx.reshape(P, NBC)
    return (np.ascontiguousarray(fidx.astype(np.int32)),
            np.ascontiguousarray(cidx.astype(np.int32)))


def feat_perm(patch, nb):
    """New feature order (kb, dx, dy8, c) -> original (c, dy, dx) column index."""
    kb, dx, dy8, c = np.meshgrid(
        np.arange(nb), np.arange(patch), np.arange(R), np.arange(C), indexing="ij"
    )
    dy = kb * R + dy8
    return (c * (patch * patch) + dy * patch + dx).reshape(-1)


def swizzle_w(wT, perm):
    """[K, D] feature-major weight -> [128, (K//128)*D], permuted to gather order."""
    w = wT[perm]
    K = w.shape[0]
    return np.ascontiguousarray(
        w.reshape(K // P, P, D).transpose(1, 0, 2).reshape(P, (K // P) * D)
    )


def make_in_maps(images, W_fine, b_fine, W_coarse, b_coarse, fine_xy, coarse_xy):
    images = np.asarray(images, dtype=np.float32)
    fine_xy = np.asarray(fine_xy, dtype=np.int64)
    coarse_xy = np.asarray(coarse_xy, dtype=np.int64)
    blob = repack_images(images)
    wf2 = swizzle_w(np.asarray(W_fine, np.float32).T.astype(BF16), feat_perm(FP, NBF))
    wc2 = swizzle_w(np.asarray(W_coarse, np.float32).T.astype(BF16), feat_perm(CP, NBC))
    bias_f = np.ascontiguousarray(np.repeat(np.asarray(b_fine, np.float32)[None, :], P, axis=0))
    bias_c = np.ascontiguousarray(np.repeat(np.asarray(b_coarse, np.float32)[None, :], P, axis=0))
    ident = np.eye(P, dtype=BF16)
    in_maps = []
    for c in range(NCORES):
        sl = slice(c * IPC, (c + 1) * IPC)
        fidx, cidx = host_indices(fine_xy[sl], coarse_xy[sl])
        in_maps.append({
            "imgs8": blob[sl].reshape(NFLAT8, 1),
            "wf2": wf2, "wc2": wc2,
            "bias_f": bias_f, "bias_c": bias_c, "ident": ident,
            "fidx": fidx, "cidx": cidx,
        })
    return in_maps


_NC_CACHE = []


def _get_nc():
    if not _NC_CACHE:
        _NC_CACHE.append(build())
    return _NC_CACHE[0]


def run(inputs: dict, trace: bool = False):
    nc = _get_nc()
    in_maps = make_in_maps(**inputs)
    res = run_bass_kernel_spmd(nc, in_maps, list(range(NCORES)), trace=trace)
    outs = [
        np.asarray(res.results[c]["out"]).reshape(IPC, NF + NCO, D)
        for c in range(NCORES)
    ]
    return np.concatenate(outs, axis=0), res


def kernel(**inputs) -> np.ndarray:
    out, _ = run(inputs, trace=False)
    return out

